# revision 9
# baseline (speedup 1.0000x reference)
"""Mindist-aware attention Trainium2 kernel (transpose-free, fully folded).

Math (per batch element b, single head, d_model = dk = 512, n = 2048).
Softmax over j kills any term constant in j, so the scores reduce to
    s'[i, j] = (x_i Ay + cy) . x_j      Ay = Wq^T Wk / sqrt(d)  (host)
                                        cy = bq Wk / sqrt(d)    (host)
and the output to
    out[i] = (sum_j p[j, i] v'[j, :]) / z[i] + bo_eff
    p[j, i] = exp(sT[j, i]) * m[level[i, j]]
    v'      = x @ (Wo Wv).T             (O-projection folded into V:
              attn @ V @ Wo.T == attn @ (X (Wo Wv).T), rows of attn sum
              to 1, bo_eff = Wo @ bv + bo absorbs the value bias)
    z[i]    = sum_j p[j, i]             (N=1 matmuls against a ones
              vector, reusing the stationary p-tile of the PV matmul)

Implementation notes:
  * Data-parallel over batch: core c computes batch element c (8 cores).
  * One projection for scores (y = x Ay + cy) and one for values (v');
    scores computed directly transposed (sT = x . y^T), so the kernel
    contains ZERO on-device transposes or casts.
  * Host passes X^T, Ay, (Wo Wv)^T in fp16 and the multiplicative level
    factors m^T = exp(bias_table - max)[level]^T pre-gathered in fp16.
    Shipping m directly (8 MB/core, streamed through a small SBUF pool)
    keeps ScalarE down to ONE activation per score tile (the exp): with
    the level->m lookup also on ScalarE it exceeds the scores-phase
    TensorE time and stalls the matmul pipeline on PSUM banks.
  * Matmuls run in fp16 with fp32 PSUM accumulation; output is written
    fp16 and upcast on the host (headroom vs the 2e-2 gate is ~30x).
  * Engine balance per score tile [128 j, 1024 i]: TensorE 8 MMs
    (~1.7us) > ScalarE exp (~1.15us) > DVE p*=m at 2 elem/cyc (~0.6us),
    so TensorE is the binding engine throughout (~143us/core of matmul
    at 78.6 TF/s fp16).
"""

import math
import os

import numpy as np

os.environ.setdefault("NEURON_FORCE_RECOMPILE", "1")
os.environ.pop("JAX_COMPILATION_CACHE_DIR", None)

N = 2048
D = 512
P = 128
NB = N // P          # 16 row blocks
DC = D // P          # 4 dim chunks
NI2 = N // 1024      # 2 i chunks of 1024

LAST_RESULT = None
LAST_NC = None
LAST_IN_MAPS = None


def build_nc(reps=1):
    return _build_bass(reps=reps)


# --------------------------------------------------------------------------
# Bass kernel
# --------------------------------------------------------------------------

def _build_bass(reps=1):
    import concourse.bacc as bacc
    import concourse.tile as tile
    import concourse.mybir as mybir

    dt = mybir.dt
    AF = mybir.ActivationFunctionType
    OP = mybir.AluOpType

    nc = bacc.Bacc("TRN2", num_devices=8)

    xt_d = nc.dram_tensor("xt", [D, N], dt.float16, kind="ExternalInput")
    mt_d = nc.dram_tensor("mt", [N, N], dt.float16, kind="ExternalInput")
    ay_d = nc.dram_tensor("ay", [D, D], dt.float16, kind="ExternalInput")
    wvo_d = nc.dram_tensor("wvot", [D, D], dt.float16, kind="ExternalInput")
    cy_d = nc.dram_tensor("cy", [D], dt.float32, kind="ExternalInput")
    bo_d = nc.dram_tensor("bo_v", [D], dt.float32, kind="ExternalInput")
    out_d = nc.dram_tensor("out", [N, D], dt.float16, kind="ExternalOutput")

    with tile.TileContext(nc) as tc:
        from contextlib import ExitStack
        with ExitStack() as ctx:
            pc = ctx.enter_context(tc.tile_pool(name="pc", bufs=1))
            pers = ctx.enter_context(tc.tile_pool(name="pers", bufs=1))
            pp = ctx.enter_context(tc.tile_pool(name="pp", bufs=2))
            pmt = ctx.enter_context(tc.tile_pool(name="pmt", bufs=8))
            pout = ctx.enter_context(tc.tile_pool(name="pout", bufs=3))
            pz = ctx.enter_context(tc.tile_pool(name="pz", bufs=4))
            ps_s = ctx.enter_context(tc.tile_pool(name="ps_s", bufs=2, space="PSUM"))
            ps_pv = ctx.enter_context(tc.tile_pool(name="ps_pv", bufs=2, space="PSUM"))

            bo_bc = pc.tile([P, D], dt.float32)
            nc.sync.dma_start(
                bo_bc[:], bo_d.rearrange("(a d) -> a d", a=1).broadcast_to([P, D]))
            cy_sb = pc.tile([P, DC], dt.float32)
            nc.sync.dma_start(cy_sb[:], cy_d.rearrange("(a p) -> p a", p=P))
            ones = pc.tile([P, 1], dt.float16)
            nc.vector.memset(ones[:], 1.0)
            warm = pc.tile([P, 512], dt.float16)
            nc.vector.memset(warm[:], 0.0)

            # persistent fp16 operands (all pre-transposed on the host)
            xt = pers.tile([P, DC, N], dt.float16)       # X^T  [d, i]
            yt = pers.tile([P, DC, N], dt.float16)       # Y^T  [d2, i]
            # V' split 257 | 256 so a ones column rides along in the first
            # half: the PV matmul then accumulates z = sum_j p[j,i] in PSUM
            # column 256 for free (no N=1 z-matmuls, no DVE reduction).
            vta = pers.tile([P, NB, 257], dt.float16)    # [V'[:, :256] | 1]
            vtb = pers.tile([P, NB, 256], dt.float16)    # V'[:, 256:]
            nc.vector.memset(vta[:, :, 256:257], 1.0)
            ayt = pers.tile([P, DC, D], dt.float16)      # Ay   [d1, d2]
            wvot = pers.tile([P, DC, D], dt.float16)     # (Wo Wv)^T [d, dm]

            for _rep in range(reps):

                # ---- loads (all contiguous, no device-side transposes) ----
                # wvot and the first xt quarter interleaved per c-chunk: the
                # V' projection's first matmul is gated on only ~256 KB.
                for c in range(DC):
                    nc.sync.dma_start(wvot[:, c, :], wvo_d[c * P:(c + 1) * P, :])
                    nc.sync.dma_start(xt[:, c, 0:512], xt_d[c * P:(c + 1) * P, 0:512])
                for q in range(1, 4):
                    qsl = slice(q * 512, (q + 1) * 512)
                    for c in range(DC):
                        nc.sync.dma_start(xt[:, c, qsl], xt_d[c * P:(c + 1) * P, qsl])
                for c in range(DC):
                    nc.sync.dma_start(ayt[:, c, :], ay_d[c * P:(c + 1) * P, :])

                # ---- PE warm-up: keep the HAM clock-gate at full rate
                # through the DMA-bound kernel prologue (zero-operand
                # matmuls into a scratch PSUM row; no consumers).
                for w in range(24):
                    wps = ps_s.tile([P, 1024], dt.float32, tag="s",
                                    name=f"warm{_rep}_{w}")
                    nc.tensor.matmul(wps[:1, :512], ones[:], warm[:],
                                     start=True, stop=True)

                # ---- projections ----
                # V': [j-chunk, dm] = sum_c X^T[d-c, j] . (Wo Wv)^T[d-c, dm]
                # (PSUM->SBUF copy on ScalarE: it is idle here, DVE is not)
                for jb in range(NB):
                    psv = ps_s.tile([P, 1024], dt.float32, tag="s",
                                    name=f"psv{_rep}_{jb}")
                    for c in range(DC):
                        nc.tensor.matmul(
                            psv[:, :512], xt[:, c, jb * P:(jb + 1) * P],
                            wvot[:, c, :], start=(c == 0), stop=(c == DC - 1))
                    nc.scalar.copy(vta[:, jb, :256], psv[:, :256])
                    nc.scalar.copy(vtb[:, jb, :], psv[:, 256:512])
                # Y^T: [d2-chunk, i] = sum_c Ay[d1-c, d2] . X^T[d1-c, i]
                for a in range(DC):
                    for ic in range(4):
                        isl = slice(ic * 512, (ic + 1) * 512)
                        psy = ps_s.tile([P, 1024], dt.float32, tag="s",
                                        name=f"psy{_rep}_{a}_{ic}")
                        for c in range(DC):
                            nc.tensor.matmul(
                                psy[:, :512], ayt[:, c, a * P:(a + 1) * P],
                                xt[:, c, isl], start=(c == 0), stop=(c == DC - 1))
                        nc.vector.tensor_scalar(
                            yt[:, a, isl], psy[:, :512], cy_sb[:, a:a + 1],
                            None, OP.add)

                # ---- attention over i-chunks of 1024 (transposed scores) ----
                for ic in range(NI2):
                    p_t = pp.tile([P, NB, 1024], dt.float16, tag="p",
                                  name=f"p{_rep}_{ic}")
                    for jb in range(NB):
                        m_t = pmt.tile([P, 1024], dt.float16, tag="m",
                                       name=f"m{_rep}_{ic}_{jb}")
                        nc.sync.dma_start(
                            m_t[:], mt_d[jb * P:(jb + 1) * P,
                                         ic * 1024:(ic + 1) * 1024])
                        ps_sT = ps_s.tile([P, 1024], dt.float32, tag="s",
                                          name=f"pss{_rep}_{ic}_{jb}")
                        for c in range(DC):  # c outer: one LDW serves 2 MMs
                            for h in range(2):
                                hs = slice(h * 512, (h + 1) * 512)
                                gis = slice(ic * 1024 + h * 512,
                                            ic * 1024 + (h + 1) * 512)
                                nc.tensor.matmul(
                                    ps_sT[:, hs], xt[:, c, jb * P:(jb + 1) * P],
                                    yt[:, c, gis], start=(c == 0),
                                    stop=(c == DC - 1))
                        nc.scalar.activation(p_t[:, jb, :], ps_sT[:], AF.Exp)
                        nc.vector.tensor_tensor(
                            p_t[:, jb, :], p_t[:, jb, :], m_t[:], OP.mult)

                    for ib in range(8):
                        ig = ic * 8 + ib
                        lsl = slice(ib * P, (ib + 1) * P)
                        # [0:257) = V' cols 0:256 + z column; [512:768) =
                        # V' cols 256:512 -- two separate PSUM banks, each
                        # accumulation group within one 2 KB bank.
                        pv = ps_pv.tile([P, 1024], dt.float32, tag="pv",
                                        name=f"pv{_rep}_{ig}")
                        for jb in range(NB):
                            nc.tensor.matmul(
                                pv[:, :257], p_t[:, jb, lsl], vta[:, jb, :],
                                start=(jb == 0), stop=(jb == NB - 1))
                            nc.tensor.matmul(
                                pv[:, 512:768], p_t[:, jb, lsl], vtb[:, jb, :],
                                start=(jb == 0), stop=(jb == NB - 1))
                        zr = pz.tile([P, 1], dt.float32, tag="zr",
                                     name=f"zr{_rep}_{ig}")
                        nc.vector.reciprocal(zr[:], pv[:, 256:257])
                        o2 = pout.tile([P, D], dt.float16, tag="o2",
                                       name=f"o2_{_rep}_{ig}")
                        nc.vector.scalar_tensor_tensor(
                            o2[:, :256], pv[:, :256], zr[:], bo_bc[:, :256],
                            OP.mult, OP.add)
                        nc.vector.scalar_tensor_tensor(
                            o2[:, 256:], pv[:, 512:768], zr[:], bo_bc[:, 256:],
                            OP.mult, OP.add)
                        nc.sync.dma_start(out_d[ig * P:(ig + 1) * P, :], o2[:])

    nc.finalize()
    return nc


def kernel(x, distance_matrix, Wq, bq, Wk, bk, Wv, bv, Wo, bo, emb_table,
           safety_threshold, _trace=False):
    global LAST_RESULT
    x = np.asarray(x, dtype=np.float32)
    distance_matrix = np.asarray(distance_matrix, np.float32)
    Wq = np.asarray(Wq, np.float32); Wk = np.asarray(Wk, np.float32)
    Wv = np.asarray(Wv, np.float32); Wo = np.asarray(Wo, np.float32)
    bq = np.asarray(bq, np.float32); bk = np.asarray(bk, np.float32)
    bv = np.asarray(bv, np.float32); bo = np.asarray(bo, np.float32)
    emb_table = np.asarray(emb_table, np.float32)
    tau = float(np.asarray(safety_threshold, np.float32))

    B, n, d = x.shape
    assert (B, n, d) == (8, N, D) and distance_matrix.shape == (8, N, N)

    # host-side scalar math (10-entry bias table -> multiplicative factors)
    w_sum = Wo.astype(np.float64).sum(axis=-1)                     # [512]
    bias_table = (emb_table.astype(np.float64) @ w_sum) / math.sqrt(D)  # [10]
    m_vals = np.exp(bias_table - bias_table.max())
    bo_eff = Wo.astype(np.float64) @ bv.astype(np.float64) + bo    # [512]
    Wvo = Wo.astype(np.float64) @ Wv.astype(np.float64)            # [512, 512]

    from concourse.bass_utils import run_bass_kernel_spmd

    nc = _build_bass()

    levels = np.clip((distance_matrix / np.float32(tau)).astype(np.int32),
                     0, 9)                                         # [8, i, j]
    mT = m_vals.astype(np.float16)[levels.transpose(0, 2, 1)]      # [8, j, i]
    s = 1.0 / math.sqrt(D)
    # y-projection fold: s'[i,j] = (x_i Ay + cy) . x_j  (q.bk / const terms
    # are constant over j and cancel in the softmax)
    Ay = (Wq.astype(np.float64).T @ Wk.astype(np.float64)) * s     # [d1, d2]
    cy = (bq.astype(np.float64) @ Wk.astype(np.float64)) * s       # [d2]
    ay_h = np.ascontiguousarray(Ay.astype(np.float16))
    wvo_h = np.ascontiguousarray(Wvo.T.astype(np.float16))         # [d, dm]
    bo_v = bo_eff.astype(np.float32)
    cy_h = cy.astype(np.float32)

    in_maps = []
    for b in range(B):
        in_maps.append({
            "xt": np.ascontiguousarray(x[b].T.astype(np.float16)),
            "mt": np.ascontiguousarray(mT[b]),
            "ay": ay_h, "wvot": wvo_h,
            "cy": cy_h, "bo_v": bo_v,
        })
    global LAST_NC, LAST_IN_MAPS
    LAST_NC, LAST_IN_MAPS = nc, in_maps
    res = run_bass_kernel_spmd(nc, in_maps, core_ids=list(range(8)),
                               trace=bool(_trace))
    LAST_RESULT = res
    out = np.stack([res.results[b]["out"] for b in range(B)], axis=0)
    return out.astype(np.float32)
